# revision 71
# baseline (speedup 1.0000x reference)
"""Trainium2 Bass kernel for nn_CAAN_84112639525649 (CAAN dense transformer).

Shapes: B=16, N=512, D_IN=256, D=64, E=32, MAXD=50.
Sharding: data-parallel over batch, 2 batches per core on 8 cores.

Key algorithmic transform: the pairwise rank-distance MLP
    rel[b,i,j] = sigmoid(relu(Eemb[clip(|r_i-r_j|,0,50)] @ Wr1 + br1) @ Wr2)
depends only on d = clip(|r_i-r_j|,0,50) in [0,50], so it collapses to a
51-entry table f[d].  With g'[k] = f(clip(|k-511|,0,50)) - f[50] (zero outside
|k-511| < 50) and Hankel band tiles H_s[p,q] = g'[128*s + p + q] (only
s in {2,3,4} intersect the band), the pairwise bias becomes
    rel[b,i,j] = f50 + sum_{v,w} R[i,v] H_{(v/128)+(w/128)}[v%128, w%128] R'[j,w]
with one-hots R[i,v] = [r_i == v], R'[j,w] = [511 - r_j == w], evaluated as
two TensorEngine matmul chains (banded: 10 + 16 matmuls per batch).
"""

import sys
import os

for _p in ("/opt/trn_rl_repo",):
    if os.path.isdir(_p) and _p not in sys.path:
        sys.path.insert(0, _p)

import numpy as np
from contextlib import ExitStack

import concourse.bass as bass
import concourse.tile as tile
from concourse.tile import add_dep_helper
from concourse import bacc, mybir
from concourse.bass import ts

N_CORES = 8
B = 16
PB = B // N_CORES  # batches per core
N = 512
D_IN = 256
D = 64
E = 32
MAXD = 50
LN_EPS = 1e-5

f32 = mybir.dt.float32
f32r = mybir.dt.float32r
bf16 = mybir.dt.bfloat16
i32 = mybir.dt.int32
AF = mybir.ActivationFunctionType
OP = mybir.AluOpType

# matmul operand mode: "f32" (exact, 4 cyc/row), "f32r" (fast fp32, 1 cyc/row),
# "bf16" (1 cyc/row, rounded operands)
MM_MODE = os.environ.get("CAAN_MM_MODE", "f32r")

# wpack layout: [128, WPACK_F] f32 staging tile packed on host.
# name -> (col, rows, cols); r rows are int32 bit-cast into f32.
WPACK = {
    "Wp0":  (0, 128, 64),
    "Wp1":  (64, 128, 64),
    "WqaT": (128, 64, 65),
    "WkaT": (1663, 64, 65),
    "Wv":   (256, 64, 64),
    "Wf1":  (320, 64, 128),
    "Wf2":  (448, 128, 64),
    "Ws1":  (512, 64, 32),
    "Ws2":  (544, 32, 1),
    "Wr1":  (545, 32, 16),
    "Wr2":  (561, 16, 1),
    "Wr2d": (562, 16, 2),
    "EembT": (1860, 32, 51),
    "bp":   (594, 64, 1),
    "bvb":  (1728, 128, 64),
    "bv":   (597, 64, 1),
    "br1":  (598, 16, 1),
    "bf1":  (599, 128, 1),
    "bs1":  (600, 32, 1),
    "bs1r": (601, 1, 32),
    "bs2":  (633, 1, 1),
    "ln_g": (634, 64, 1),
    "ln_b": (635, 64, 1),
    "bf2b": (636, 128, 64),
    "r0":   (700, 1, 512),
    "r1":   (1212, 1, 512),
}
WPACK_F = 1792


def _st():
    if MM_MODE == "bf16":
        return bf16
    if MM_MODE == "f32r":
        return f32r
    return f32


def build_nc(mm_mode=None):
    global MM_MODE
    if mm_mode is not None:
        MM_MODE = mm_mode
    ST = _st()

    nc = bacc.Bacc(
        "TRN2",
        target_bir_lowering=False,
        debug=False,
        enable_asserts=False,
        num_devices=N_CORES,
    )

    # ---- DRAM I/O ----
    x_d = nc.dram_tensor("x", (PB, N, D_IN), f32, kind="ExternalInput")
    wp_d = nc.dram_tensor("wpack", (128, WPACK_F), f32, kind="ExternalInput")
    out_d = nc.dram_tensor("out", (PB, N), f32, kind="ExternalOutput")
    g_d = nc.dram_tensor("gline", (2 * N - 1,), _st(), kind="Internal")

    x_ap = x_d.ap()
    out_ap = out_d.ap()

    with ExitStack() as ctx:
        tc = ctx.enter_context(tile.TileContext(nc))
        cp = ctx.enter_context(tc.tile_pool(name="consts", bufs=1))
        wk = ctx.enter_context(tc.tile_pool(name="work", bufs=1))
        ps = ctx.enter_context(tc.tile_pool(name="ps", bufs=5, space="PSUM"))

        PS_BUFS = {"mm": 4, "io": 2, "sm": 2}

        def stcast(ap):
            # f32r memsets are invalid ISA; the bit pattern matches f32
            return ap.bitcast(f32) if ST == f32r else ap

        def psum(shape, tag="mm", dtype=f32, bufs=None):
            if bufs is None:
                bufs = PS_BUFS[tag]
            return ps.tile(shape, dtype, tag=tag, name="pst", bufs=bufs)

        # ============ input DMAs first ============
        # weights pack first: the f->g'->H chain hangs off it
        wpk = cp.tile([128, WPACK_F], f32, tag="wpk")
        HALF = 864
        nc.sync.dma_start(wpk[:, :HALF], wp_d.ap()[:, :HALF])
        nc.scalar.dma_start(wpk[:, HALF:], wp_d.ap()[:, HALF:])
        # x: one 3D DMA per batch -> xall[b] [128 tok, 4 tchunk, 256 din]
        xall = []
        for b in range(PB):
            xt = wk.tile([128, 4, D_IN], f32, tag=f"xall{b}", name="xall")
            nc.sync.dma_start(
                xt[:, :, :],
                bass.AP(tensor=x_d, offset=b * N * D_IN,
                        ap=[[D_IN, 128], [128 * D_IN, 4], [1, D_IN]]),
            )
            xall.append(xt)

        def wsl(name):
            col, rows, cols = WPACK[name]
            return wpk[0:rows, col : col + cols]

        # identities for PE transposes (iota + is_equal keeps Pool free)
        ident_io = cp.tile([128, 128], i32, tag="ident_io")
        nc.gpsimd.iota(ident_io[:, :], pattern=[[-1, 128]], base=0,
                       channel_multiplier=1)
        ident = cp.tile([128, 128], f32, tag="ident")
        nc.vector.tensor_scalar(ident[:, :], ident_io[:, :], 0, None,
                                op0=OP.is_equal)
        if ST != f32:
            ident_st = cp.tile([128, 128], ST, tag="ident_st")
            nc.vector.tensor_scalar(ident_st[:, :], ident_io[:, :], 0, None,
                                    op0=OP.is_equal)
        else:
            ident_st = ident

        bp_c = wsl("bp")
        bv_b = wsl("bvb")
        br1_c = wsl("br1")
        bf1_c = wsl("bf1")
        bs1_r = wsl("bs1r")
        bs2_c = wsl("bs2")
        lng_c = wsl("ln_g")
        lnb_c = wsl("ln_b")
        bf2_b = wsl("bf2b")

        # ---- f table: f[d] = sigmoid(relu(Eemb[d] @ Wr1 + br1) @ Wr2) ----
        m1T_ps = psum([16, MAXD + 1], tag="io")
        nc.tensor.matmul(m1T_ps[:, :], wsl("Wr1"), wsl("EembT"),
                         start=True, stop=True)
        m1T = cp.tile([16, MAXD + 1], f32, tag="m1T")
        crit = {}
        crit["m1T_mm"] = nc.scalar.activation(m1T[:, :], m1T_ps[:, :], AF.Relu,
                                              bias=br1_c, scale=1.0)
        # f as a row, computed directly (lhsT = Wr2) to skip a transpose hop
        frow_ps = psum([1, MAXD + 1], tag="io")
        nc.tensor.matmul(frow_ps[:, :], wsl("Wr2"), m1T[:16, : MAXD + 1],
                         start=True, stop=True)
        f_row = cp.tile([1, MAXD + 1], f32, tag="f_row")
        nc.scalar.activation(f_row[:, :], frow_ps[:, :], AF.Sigmoid)
        f_row32 = f_row
        # ---- g' line: g'[k] = f[min(|k-511|,50)] - f[50], k in [0, 768) ----
        # (H tiles read only g'[256..767).  One-hot Oa/Ob [64, 768] with the
        # far-field folded into row 50 as an is_ge band; rows 51..63 are
        # killed by zero rows of f64.  All builds are f-independent.
        NG0 = 768
        lo = N - 1 - MAXD  # 461
        wdt = 2 * MAXD + 2  # 102 cols [461, 563)
        Oa_t = cp.tile([64, NG0], ST, tag="Oa")
        nc.vector.memset(stcast(Oa_t[:, :]), 0.0)
        Ob_t = cp.tile([64, NG0], ST, tag="Ob")
        nc.vector.memset(stcast(Ob_t[:, :]), 0.0)
        iota_a = cp.tile([MAXD, wdt], i32, tag="iota_a")  # k-511-t, k=461+col
        nc.gpsimd.iota(iota_a[:, :], pattern=[[1, wdt]], base=-MAXD,
                       channel_multiplier=-1)
        iota_b = cp.tile([MAXD, wdt], i32, tag="iota_b")  # 511-k-t
        nc.gpsimd.iota(iota_b[:, :], pattern=[[-1, wdt]], base=MAXD,
                       channel_multiplier=-1)
        nga = NG0 - (N + MAXD - 1)  # 207 cols [561, 768)
        iota_ga = cp.tile([32, nga], i32, tag="iota_ga")  # k-511-t, t=32+p
        nc.gpsimd.iota(iota_ga[:, :], pattern=[[1, nga]], base=MAXD - 32,
                       channel_multiplier=-1)
        ngb = N - MAXD  # 462 cols [0, 462)
        iota_gb = cp.tile([32, ngb], i32, tag="iota_gb")  # 511-k-t, t=32+p
        nc.gpsimd.iota(iota_gb[:, :], pattern=[[-1, ngb]], base=N - 33,
                       channel_multiplier=-1)
        # is_ge band rows [32:64) first, then is_eq rows [0:50) overwrite,
        # then zero the is_ge spill in rows [32:50)
        nc.vector.tensor_scalar(Oa_t[32:64, N + MAXD - 1 : NG0], iota_ga[:, :],
                                0, None, op0=OP.is_ge)
        nc.vector.tensor_scalar(Oa_t[:MAXD, lo : lo + wdt], iota_a[:, :],
                                0, None, op0=OP.is_equal)
        nc.vector.memset(stcast(Oa_t[32:50, N + MAXD + 1 : NG0]), 0.0)
        nc.vector.memset(stcast(Oa_t[0:1, N - 1 : N]), 0.0)  # k=511 overlap
        nc.vector.tensor_scalar(Ob_t[32:64, 0:ngb], iota_gb[:, :],
                                0, None, op0=OP.is_ge)
        nc.vector.tensor_scalar(Ob_t[:MAXD, lo : lo + wdt], iota_b[:, :],
                                0, None, op0=OP.is_equal)
        nc.vector.memset(stcast(Ob_t[32:50, 0:lo]), 0.0)

        # f64 [64, 2]: rows 0..50 = f (duplicated columns), rows 51..63 = 0
        f64 = cp.tile([64, 2], ST, tag="f64")
        nc.vector.memset(stcast(f64[:, :]), 0.0)
        fcol_ps = psum([MAXD + 1, 2], tag="io")
        nc.tensor.matmul(fcol_ps[:, :], m1T[:16, : MAXD + 1], wsl("Wr2d"),
                         start=True, stop=True)
        crit["f64sig"] = nc.scalar.activation(f64[: MAXD + 1, :],
                                              fcol_ps[:, :], AF.Sigmoid)

        g_ps0 = psum([2, N], tag="io")
        nc.tensor.matmul(g_ps0[:, :], f64[:, :], Oa_t[:, :N],
                         start=True, stop=False)
        nc.tensor.matmul(g_ps0[:, :], f64[:, :], Ob_t[:, :N],
                         start=False, stop=True)
        g_ps1 = psum([2, NG0 - N], tag="io")
        nc.tensor.matmul(g_ps1[:, :], f64[:, :], Oa_t[:, N:NG0],
                         start=True, stop=False)
        crit["gmm"] = nc.tensor.matmul(g_ps1[:, :], f64[:, :],
                                       Ob_t[:, N:NG0], start=False, stop=True)
        g_sb = cp.tile([1, NG0], ST, tag="g_sb")
        nc.vector.tensor_scalar(g_sb[:, :N], g_ps0[0:1, :],
                                f_row32[0:1, MAXD : MAXD + 1], None,
                                op0=OP.subtract)
        crit["gsb"] = nc.vector.tensor_scalar(g_sb[:, N:NG0], g_ps1[0:1, :],
                                              f_row32[0:1, MAXD : MAXD + 1],
                                              None, op0=OP.subtract)
        nc.sync.dma_start(
            bass.AP(tensor=g_d, offset=0, ap=[[1, NG0]]), g_sb[0:1, :])

        # Hankel band tiles H_s[p,q] = g'[128*s + p + q], s in {2,3,4}
        H_t = {}
        for s in (2, 3, 4):
            ht = cp.tile([128, 128], ST, tag=f"H{s}", name="ht")
            eng = nc.sync if s == 2 else nc.scalar
            eng.dma_start(
                ht[:, :],
                bass.AP(tensor=g_d, offset=128 * s, ap=[[1, 128], [1, 128]]),
            )
            H_t[s] = ht
        BANDS = {0: (2, 3), 1: (1, 2, 3), 2: (0, 1, 2), 3: (0, 1)}

        # f50 broadcast column (for adding back the far-field constant)
        f50_c = cp.tile([128, 1], f32, tag="f50c")
        nc.gpsimd.partition_broadcast(f50_c[:, :], f_row32[0:1, MAXD : MAXD + 1])


        # matmul weights converted to ST (direct slices when ST == f32)
        def conv_w(name):
            col, rows, cols = WPACK[name]
            if ST == f32:
                return wsl(name)
            t = cp.tile([rows, cols], ST, tag=f"cw_{name}", name="cw")
            nc.gpsimd.tensor_copy(t[:, :], wsl(name))
            return t[0:rows, 0:cols]

        Wp_t = [conv_w("Wp0"), conv_w("Wp1")]
        Wv_t = conv_w("Wv")
        Wf1_t = conv_w("Wf1")
        Wf2_t = conv_w("Wf2")
        Ws2_t = conv_w("Ws2")

        # M1 = [Wq;bq] [Wk;bk]^T / 8 : the whole score bilinear form [65,65]
        M1_ps = psum([D + 1, D + 1], tag="io")
        nc.tensor.matmul(M1_ps[:, :], wsl("WqaT"), wsl("WkaT"),
                         start=True, stop=True)
        M1_t = cp.tile([D + 1, D + 2], ST, tag="M1")
        nc.vector.memset(stcast(M1_t[:, D + 1 : D + 2]), 0.0)
        nc.vector.tensor_scalar(M1_t[:, : D + 1], M1_ps[:, :], 0.125, None,
                                op0=OP.mult)

        # Ws1' = diag(ln_g) @ Ws1 ; bs1' = ln_b @ Ws1 + bs1
        Ws1p = cp.tile([D, 32], _st(), tag="Ws1p")
        nc.vector.tensor_scalar(Ws1p[:, :], wsl("Ws1"), lng_c, None, op0=OP.mult)
        bs1p_ps = psum([1, 32], tag="io")
        nc.tensor.matmul(bs1p_ps[:, :], lnb_c, wsl("Ws1"), start=True, stop=True)
        bs1p_row = cp.tile([1, 32], f32, tag="bs1p_row")
        nc.vector.tensor_add(bs1p_row[:, :], bs1p_ps[:, :], bs1_r)
        bs1p_tps = psum([32, 1], tag="io")
        nc.tensor.transpose(bs1p_tps[:, :], bs1p_row[0:1, :32], ident[0:1, 0:1])
        bs1p_c = cp.tile([32, 1], f32, tag="bs1p")
        nc.vector.tensor_copy(bs1p_c[:, :], bs1p_tps[:, :])

        # per-chunk iota columns (f32) for one-hot compares
        iota_cols = []
        for c in range(4):
            it_i = cp.tile([128, 1], i32, tag=f"iota_i{c}", name="ii")
            nc.gpsimd.iota(it_i[:, :], pattern=[[0, 1]], base=c * 128,
                           channel_multiplier=1)
            it_f = cp.tile([128, 1], f32, tag=f"iota_f{c}", name="if")
            nc.gpsimd.tensor_copy(it_f[:, :], it_i[:, :])
            iota_cols.append(it_f)

        # eps column for layernorm
        eps_c = cp.tile([128, 1], f32, tag="eps")
        nc.vector.memset(eps_c[:, :], LN_EPS)
        # negated output bias (sigmoid computed as 1/(1+exp(-x)))
        nbs2_c = cp.tile([1, 1], f32, tag="nbs2")
        nc.vector.tensor_scalar_mul(nbs2_c[:, :], bs2_c, -1.0)

        # ================= per-batch (stages interleaved across batches) ====
        st_ = [dict() for _ in range(PB)]

        def stage_onehot(b):
            S = st_[b]
            col, _, _ = WPACK[f"r{b}"]
            rf_row = wk.tile([1, N], f32, tag=f"rf_row{b}", name="rfr")
            nc.vector.tensor_copy(rf_row[:, :],
                                  wpk[0:1, col : col + N].bitcast(i32))
            rp_row = wk.tile([1, N], f32, tag=f"rp_row{b}", name="rpr")
            nc.vector.tensor_scalar(rp_row[:, :], rf_row[0:1, :], -1.0,
                                    float(N - 1), op0=OP.mult, op1=OP.add)
            r_bc = wk.tile([128, N], f32, tag=f"r_bc{b}", name="rbc")
            nc.gpsimd.partition_broadcast(r_bc[:, :], rf_row[0:1, :])
            rp_bc = wk.tile([128, N], f32, tag=f"rp_bc{b}", name="rpbc")
            nc.gpsimd.partition_broadcast(rp_bc[:, :], rp_row[0:1, :])
            S["RT"] = []
            S["RpT"] = []
            for c in range(4):
                rpt = wk.tile([128, N], ST, tag=f"RpT{b}_{c}", name="rpt")
                nc.gpsimd.tensor_scalar(rpt[:, :], rp_bc[:, :],
                                        iota_cols[c][:, 0:1], None,
                                        op0=OP.is_equal)
                S["RpT"].append(rpt)
                rt = wk.tile([128, N], ST, tag=f"RT{b}_{c}", name="rt")
                rti = nc.vector.tensor_scalar(rt[:, :], r_bc[:, :],
                                              iota_cols[c][:, 0:1], None,
                                              op0=OP.is_equal)
                if b == 0 and c == 0:
                    crit["rt00"] = rti
                S["RT"].append(rt)

        def stage_xp(b):
            S = st_[b]
            xT = [wk.tile([128, N], ST, tag=f"xT{b}_{d}", name="xT")
                  for d in range(2)]
            for d in range(2):
                xT_ps = psum([128, N])
                for t in range(4):
                    ti = nc.tensor.transpose(
                        xT_ps[:, ts(t, 128)], xall[b][:, t, ts(d, 128)],
                        ident[:, :]
                    )
                    if b == 0 and d == 0 and t == 0:
                        crit["xt0"] = ti
                ci = nc.scalar.copy(xT[d][:, :], xT_ps[:, :])
                if b == 0 and d == 0:
                    crit["xtd0"] = ci
            xpT_ps = psum([D, N], tag="io")
            for d in range(2):
                nc.tensor.matmul(xpT_ps[:, :], Wp_t[d], xT[d][:, :],
                                 start=(d == 0), stop=(d == 1))
            xpT = wk.tile([D + 1, N], ST, tag=f"xpT{b}", name="xpT")
            nc.vector.tensor_scalar(xpT[:D, :], xpT_ps[:, :], bp_c, None,
                                    op0=OP.add)
            nc.vector.memset(stcast(xpT[D : D + 1, :]), 1.0)
            S["xpT"] = xpT

        def stage_uv(b):
            S = st_[b]
            xpT = S["xpT"]
            uT_ps = psum([D + 2, N], tag="io")
            nc.tensor.matmul(uT_ps[:, :], M1_t[:, :], xpT[:, :],
                             start=True, stop=True)
            uT = wk.tile([D + 2, N], ST, tag=f"uT{b}", name="uT")
            nc.scalar.copy(uT[:, :], uT_ps[:, :])
            S["uT"] = uT
            S["v_sb"] = []
            for c in range(4):
                v_ps = psum([128, D], tag="sm")
                nc.tensor.matmul(v_ps[:, :], xpT[:D, ts(c, 128)], Wv_t,
                                 start=True, stop=True)
                vs = wk.tile([128, D], ST, tag=f"v{b}_{c}", name="vs")
                nc.vector.tensor_add(vs[:, :], v_ps[:, :], bv_b)
                S["v_sb"].append(vs)

        def stage_t1(b):
            S = st_[b]
            S["rr_ps"] = psum([1, N], tag="io")
            S["T1"] = []
            for c in range(4):
                t1p = psum([128, N])
                ws = BANDS[c]
                for wi, w in enumerate(ws):
                    nc.tensor.matmul(t1p[:, :], H_t[c + w][:, :],
                                     S["RpT"][w][:, :], start=(wi == 0),
                                     stop=(wi == len(ws) - 1))
                t1s = wk.tile([128, N], ST, tag=f"T1{b}_{c}", name="t1s")
                if c < 2:
                    nc.vector.tensor_copy(t1s[:, :], t1p[:, :])
                else:
                    nc.scalar.copy(t1s[:, :], t1p[:, :])
                S["T1"].append(t1s)

        def stage_attn(b, c):
            S = st_[b]
            sp = psum([128, N])
            nc.tensor.matmul(sp[:, :], S["uT"][: D + 1, ts(c, 128)],
                             S["xpT"][:, :], start=True, stop=True)
            ssb = wk.tile([128, N], f32, tag=f"S{b}_{c}", name="ssb")
            if c < 2:
                nc.scalar.copy(ssb[:, :], sp[:, :])
            else:
                nc.vector.tensor_copy(ssb[:, :], sp[:, :])
            relp = psum([128, N])
            for v in range(4):
                nc.tensor.matmul(relp[:, :], S["RT"][v][:, ts(c, 128)],
                                 S["T1"][v][:, :], start=(v == 0),
                                 stop=(v == 3))
            p_st = wk.tile([128, N], ST, tag=f"P{b}_{c}", name="pst")
            nc.vector.scalar_tensor_tensor(p_st[:, :], relp[:, :],
                                           f50_c[:, 0:1], ssb[:, :],
                                           op0=OP.add, op1=OP.mult)
            sums = wk.tile([128, 1], f32, tag=f"sum{b}_{c}", name="sums")
            nc.scalar.activation(p_st[:, :], p_st[:, :], AF.Exp,
                                 accum_out=sums[:, 0:1])
            rc = wk.tile([128, 1], f32, tag=f"rec{b}_{c}", name="rc")
            nc.vector.reciprocal(rc[:, :], sums[:, 0:1])
            nc.tensor.transpose(S["rr_ps"][:, ts(c, 128)], rc[:, 0:1],
                                ident[:, :])
            S.setdefault("A", []).append(p_st)

        def stage_av(b):
            S = st_[b]
            AT = []
            for j in range(4):
                atp = psum([128, N], dtype=ST)
                for c in range(4):
                    nc.tensor.transpose(atp[:, ts(c, 128)],
                                        S["A"][c][:, ts(j, 128)],
                                        ident_st[:, :])
                ats = wk.tile([128, N], ST, tag=f"AT{b}_{j}", name="ats")
                if j % 2 == 0:
                    nc.vector.tensor_copy(ats[:, :], atp[:, :])
                else:
                    nc.scalar.copy(ats[:, :], atp[:, :])
                AT.append(ats)
            rr_sb = wk.tile([1, N], f32, tag=f"rr{b}", name="rr")
            nc.vector.tensor_copy(rr_sb[:, :], S["rr_ps"][:, :])
            rec_bc = wk.tile([128, N], f32, tag=f"recbc{b}", name="recbc")
            nc.gpsimd.partition_broadcast(rec_bc[:, :], rr_sb[0:1, :])
            aoT_ps = psum([D, N], tag="io")
            for j in range(4):
                nc.tensor.matmul(aoT_ps[:, :], S["v_sb"][j][:, :], AT[j][:, :],
                                 start=(j == 0), stop=(j == 3))
            aoT = wk.tile([D, N], ST, tag=f"aoT{b}", name="aoT")
            nc.vector.scalar_tensor_tensor(aoT[:, :], aoT_ps[:, :], 1.0,
                                           rec_bc[:D, :], op0=OP.mult,
                                           op1=OP.mult)
            S["aoT"] = aoT

        def stage_ffn(b):
            S = st_[b]
            h1T_ps = psum([2 * D, N], tag="io")
            nc.tensor.matmul(h1T_ps[:, :], Wf1_t, S["aoT"][:, :], start=True,
                             stop=True)
            h1T = wk.tile([2 * D, N], ST, tag=f"h1T{b}", name="h1T")
            nc.scalar.activation(h1T[:, :], h1T_ps[:, :], AF.Relu,
                                 bias=bf1_c, scale=1.0)
            zT_ps = psum([D, N], tag="io", dtype=ST)
            for c in range(4):
                h_ps = psum([128, D], tag="sm")
                nc.tensor.matmul(h_ps[:, :], h1T[:, ts(c, 128)], Wf2_t,
                                 start=True, stop=True)
                h_sb = wk.tile([128, D], f32, tag=f"h{b}_{c}", name="hsb")
                nc.vector.tensor_add(h_sb[:, :], h_ps[:, :], bf2_b)
                stats = wk.tile([128, 6], f32, tag=f"st{b}_{c}", name="sts")
                nc.vector.bn_stats(stats[:, :], h_sb[:, :])
                mv = wk.tile([128, 2], f32, tag=f"mv{b}_{c}", name="mv")
                nc.vector.bn_aggr(mv[:, :], stats[:, :])
                std = wk.tile([128, 1], f32, tag=f"std{b}_{c}", name="std")
                nc.scalar.activation(std[:, :], mv[:, 1:2], AF.Sqrt,
                                     bias=eps_c[:, 0:1], scale=1.0)
                rstd = wk.tile([128, 1], f32, tag=f"rstd{b}_{c}", name="rstd")
                nc.vector.reciprocal(rstd[:, :], std[:, :])
                z_sb = wk.tile([128, D], ST, tag=f"z{b}_{c}", name="zsb")
                nc.vector.tensor_scalar(z_sb[:, :], h_sb[:, :], mv[:, 0:1],
                                        rstd[:, 0:1], op0=OP.subtract,
                                        op1=OP.mult)
                nc.tensor.transpose(zT_ps[:, ts(c, 128)], z_sb[:, :D],
                                    ident_st[:, :])
            zT = wk.tile([D, N], ST, tag=f"zT{b}", name="zT")
            nc.vector.tensor_copy(zT[:, :], zT_ps[:, :])
            s1T_ps = psum([32, N], tag="io")
            nc.tensor.matmul(s1T_ps[:, :], Ws1p[:D, :32], zT[:, :],
                             start=True, stop=True)
            s1T = wk.tile([32, N], ST, tag=f"s1T{b}", name="s1T")
            nc.scalar.activation(s1T[:, :], s1T_ps[:, :], AF.Relu,
                                 bias=bs1p_c[:32, 0:1], scale=1.0)
            o_ps = psum([1, N], tag="io")
            nc.tensor.matmul(o_ps[:, :], Ws2_t, s1T[:, :], start=True, stop=True)
            o_sb = wk.tile([1, N], f32, tag=f"o_sb{b}", name="osb")
            nc.scalar.activation(o_sb[:, :], o_ps[:, :], AF.Exp,
                                 bias=nbs2_c[0:1, 0:1], scale=-1.0)
            nc.vector.tensor_scalar(o_sb[:, :], o_sb[:, :], 1.0, None,
                                    op0=OP.add)
            nc.vector.reciprocal(o_sb[:, :], o_sb[:, :])
            nc.scalar.dma_start(out_ap[b : b + 1, :], o_sb[0:1, :])

        # interleaved emission: batch stages alternate so both fill gaps
        stage_onehot(0)
        stage_xp(0)
        stage_onehot(1)
        stage_xp(1)
        for b in range(PB):
            stage_uv(b)
        stage_t1(0)
        for c in range(4):
            stage_attn(0, c)
        stage_t1(1)
        stage_av(0)
        for c in range(4):
            stage_attn(1, c)
        stage_ffn(0)
        stage_av(1)
        stage_ffn(1)

        # force the f->g'->H critical chain ahead of batch work per engine
        add_dep_helper(crit["xtd0"].ins, crit["f64sig"].ins,
                       sync=False, reason="xT drains after f sigmoids")
        add_dep_helper(crit["rt00"].ins, crit["gsb"].ins,
                       sync=False, reason="RT builds after g drain")

    nc.compile()
    return nc


def _pack_weights(inputs, r):
    w = np.zeros((128, WPACK_F), np.float32)

    def put(name, arr):
        col, rows, cols = WPACK[name]
        a = np.asarray(arr, np.float32).reshape(rows, cols)
        w[0:rows, col : col + cols] = a

    Wp = np.asarray(inputs["Wp"], np.float32)
    put("Wp0", Wp[:128])
    put("Wp1", Wp[128:])
    Wqa = np.concatenate([np.asarray(inputs["Wq"], np.float32),
                          np.asarray(inputs["bq"], np.float32).reshape(1, D)], 0)
    Wka = np.concatenate([np.asarray(inputs["Wk"], np.float32),
                          np.asarray(inputs["bk"], np.float32).reshape(1, D)], 0)
    put("WqaT", Wqa.T)
    put("WkaT", Wka.T)
    put("Wv", inputs["Wv"])
    put("Wf1", inputs["Wf1"])
    put("Wf2", inputs["Wf2"])
    put("Ws1", inputs["Ws1"])
    put("Ws2", inputs["Ws2"])
    put("Wr1", inputs["Wr1"])
    put("Wr2", inputs["Wr2"])
    put("Wr2d", np.concatenate([np.asarray(inputs["Wr2"], np.float32)] * 2, 1))
    put("EembT", np.asarray(inputs["Eemb"], np.float32).T)
    put("bp", inputs["bp"])
    put("bvb", np.broadcast_to(np.asarray(inputs["bv"], np.float32), (128, D)))
    put("br1", inputs["br1"])
    put("bf1", inputs["bf1"])
    put("bs1", inputs["bs1"])
    put("bs1r", inputs["bs1"])
    put("bs2", inputs["bs2"])
    put("ln_g", inputs["ln_g"])
    put("ln_b", inputs["ln_b"])
    put("bf2b", np.broadcast_to(np.asarray(inputs["bf2"], np.float32),
                                (128, D)))
    for b in range(PB):
        col, _, _ = WPACK[f"r{b}"]
        w[0, col : col + N] = r[b].view(np.float32)
    return w


_NC_CACHE = {}


def _get_nc(mm_mode=None):
    key = mm_mode or MM_MODE
    if key not in _NC_CACHE:
        _NC_CACHE[key] = build_nc(key)
    return _NC_CACHE[key]


def kernel(**inputs):
    from concourse.bass_utils import run_bass_kernel_spmd

    nc = _get_nc()

    x = np.ascontiguousarray(np.asarray(inputs["x"], dtype=np.float32))
    r = np.asarray(inputs["price_rising_ranks"]).astype(np.int32)
    assert x.shape == (B, N, D_IN)

    in_maps = []
    for c in range(N_CORES):
        rloc = np.ascontiguousarray(r[c * PB : (c + 1) * PB])
        in_maps.append({
            "x": np.ascontiguousarray(x[c * PB : (c + 1) * PB]),
            "wpack": _pack_weights(inputs, rloc),
        })

    res = run_bass_kernel_spmd(nc, in_maps, core_ids=list(range(N_CORES)))
    out = np.concatenate([res.results[c]["out"] for c in range(N_CORES)], axis=0)
    return out.astype(np.float32)
